# revision 16
# baseline (speedup 1.0000x reference)
"""LSG (local-sparse-global) block-local self-attention for Trainium2.

Problem: n=2, h=16, t=4096, d=64, block=128. Each query block attends to a
3-block local key window (1-block halo each side) plus a global BOS token
slot; the BOS query (position 0) attends to everything.

Strategy (8 NeuronCores, batch*head = 32 sharded 4 per core):
  - Host pre-transposes Q/K to [d, t] bf16 layouts and appends a ones-column
    to V so per-query softmax denominators ride along the PV matmul.
  - Device computes, per key block j, S^T = kT_j.T @ qT_union in PSUM
    (two key blocks run concurrently via PE row tiling: kT_j on partitions
    0-63, kT_{j+1} on 64-127, with qT duplicated on both halves).
  - Softmax uses a constant bias instead of a running max:
    p = exp(s/8 - 50). Scores are O(+-40) so exp stays in fp32 normal range
    and the bias cancels after normalization. This removes the max-reduce
    and, crucially, means exp output IS already P^T (keys on partitions), so
    the PV matmul needs no transpose of P.
  - out^T[d, q] (+ sums row 64) accumulates over the 3 window key blocks.
  - Host divides by sums, adds the BOS-token key slot for query blocks >= 2
    (for blocks 0/1 key 0 is already inside the local window, which matches
    the reference's global-slot semantics exactly), and computes the single
    BOS query row. These host pieces are ~0.5% of total FLOPs.
"""

import sys

import numpy as np
import ml_dtypes

try:  # concourse (bass) ships in the trn_rl repo, not on the default path
    import concourse.bass  # noqa: F401
except ImportError:
    for _p in ("/opt/trn_rl_repo", "/root/.axon_site/_ro/trn_rl_repo"):
        if _p not in sys.path:
            sys.path.insert(0, _p)

N, H, T, D = 2, 16, 4096, 64
BLOCK = 128
NB = T // BLOCK            # 32 key/query blocks
BH = N * H                 # 32 batch*head pairs
NCORES = 8
BH_PER_CORE = BH // NCORES  # 4
GUARD_NB = NB + 2          # query column blocks incl. zero guards
EXP_BIAS = 0.0             # scores/8 ~ N(0,1): plain exp stays in fp32 range
SCALE = 1.0 / 8.0          # 1/sqrt(64)

_BF16 = ml_dtypes.bfloat16

_CACHE = {}


def _build_bass():
    import concourse.bacc as bacc
    import concourse.mybir as mybir
    import concourse.tile as tile

    bf16 = mybir.dt.bfloat16
    f32 = mybir.dt.float32

    nc = bacc.Bacc(None, target_bir_lowering=False)
    qt = nc.declare_dram_parameter(
        "qt", [BH_PER_CORE, 128, GUARD_NB * BLOCK], bf16, isOutput=False
    )
    kt = nc.declare_dram_parameter("kt", [BH_PER_CORE, 64, T], bf16, isOutput=False)
    va = nc.declare_dram_parameter("va", [BH_PER_CORE, T, D + 1], bf16, isOutput=False)
    out = nc.declare_dram_parameter(
        "out", [BH_PER_CORE, NB, D + 1, BLOCK], f32, isOutput=True
    )

    with tile.TileContext(nc) as tc:
        with (
            tc.tile_pool(name="cst", bufs=1) as cst,
            tc.tile_pool(name="sbq", bufs=2) as sbq,
            tc.tile_pool(name="sbk", bufs=3) as sbk,
            tc.tile_pool(name="sbv", bufs=3) as sbv,
            tc.tile_pool(name="sbp", bufs=1) as sbp,
            tc.tile_pool(name="sbo", bufs=3) as sbo,
            tc.tile_pool(name="psS", bufs=4, space="PSUM") as psS,
            tc.tile_pool(name="psO", bufs=3, space="PSUM") as psO,
        ):
            bias_tile = cst.tile([128, 1], f32, tag="bias")
            nc.vector.memset(bias_tile, EXP_BIAS)
            # Touch the bias from ACT once so later Exp ops don't each carry
            # a cross-engine wait (the AC instruction has one wait slot).
            warm = cst.tile([128, 1], f32, tag="warm")
            nc.scalar.activation(
                out=warm,
                in_=bias_tile,
                func=mybir.ActivationFunctionType.Copy,
                bias=0.0,
                scale=1.0,
            )
            for bh in range(BH_PER_CORE):
                qta = sbq.tile([128, GUARD_NB * BLOCK], bf16, tag="qta")
                # load the qT panel in 4 chunks so compute can start early
                qchunk = GUARD_NB * BLOCK // 4
                for c in range(4):
                    nc.sync.dma_start(
                        out=qta[:, c * qchunk : (c + 1) * qchunk],
                        in_=qt[bh, :, c * qchunk : (c + 1) * qchunk],
                    )

                pts = {}
                vas = {}
                for p in range(NB // 2):
                    j0, j1 = 2 * p, 2 * p + 1
                    kp = sbk.tile([128, BLOCK], bf16, tag="kp")
                    nc.sync.dma_start(
                        out=kp[0:64, :], in_=kt[bh, :, j0 * BLOCK : (j0 + 1) * BLOCK]
                    )
                    nc.sync.dma_start(
                        out=kp[64:128, :], in_=kt[bh, :, j1 * BLOCK : (j1 + 1) * BLOCK]
                    )
                    for j in (j0, j1):
                        vtile = sbv.tile(
                            [128, D + 1], bf16, tag=f"va{j % 2}", name=f"va_{bh}_{j}"
                        )
                        nc.sync.dma_start(
                            out=vtile, in_=va[bh, j * BLOCK : (j + 1) * BLOCK, :]
                        )
                        vas[j] = vtile

                    # scores^T for the pair: union of query blocks j0-1..j0+2
                    u = j0 * BLOCK  # qta column of query block j0-1 (+1 guard shift)
                    sA = psS.tile([128, 512], f32, tag="spair")
                    sB = psS.tile([128, 512], f32, tag="spair")
                    nc.tensor.matmul(
                        out=sA[:, :],
                        lhsT=kp[0:64, :],
                        rhs=qta[0:64, u : u + 512],
                        start=True,
                        stop=True,
                    )
                    nc.tensor.matmul(
                        out=sB[:, :],
                        lhsT=kp[64:128, :],
                        rhs=qta[64:128, u : u + 512],
                        start=True,
                        stop=True,
                    )

                    for j, sP in ((j0, sA), (j1, sB)):
                        jlo, jhi = max(0, j - 1), min(NB - 1, j + 1)
                        c0 = (jlo - (j0 - 1)) * BLOCK
                        w = (jhi - jlo + 1) * BLOCK
                        # unique slot per tile: pt slots are never reused, so
                        # Exp ops never need a WAW wait (AC has 1 wait slot)
                        ptj = sbp.tile(
                            [128, 3 * BLOCK],
                            bf16,
                            tag=f"ptj_{bh}_{j}",
                            name=f"pt_{bh}_{j}",
                        )
                        nc.scalar.activation(
                            out=ptj[:, 0:w],
                            in_=sP[:, c0 : c0 + w],
                            func=mybir.ActivationFunctionType.Exp,
                            bias=bias_tile[:, :],
                            scale=SCALE,
                        )
                        pts[j] = (ptj, jlo)

                    # query blocks whose full window (keys i-1..i+1) is now
                    # available: run their 3 accumulating matmuls
                    # back-to-back so each PSUM group opens and closes
                    # immediately, then copy out.
                    if p == 0:
                        done = [0]
                    elif p == NB // 2 - 1:
                        done = [2 * p - 1, 2 * p, 2 * p + 1]
                    else:
                        done = [2 * p - 1, 2 * p]
                    for i in done:
                        ilo, ihi = max(0, i - 1), min(NB - 1, i + 1)
                        acc = psO.tile(
                            [D + 1, BLOCK], f32, tag="acc", name=f"acc_{bh}_{i}"
                        )
                        for j in range(ilo, ihi + 1):
                            ptj, jlo = pts[j]
                            nc.tensor.matmul(
                                out=acc[:, :],
                                lhsT=vas[j][:, :],
                                rhs=ptj[:, (i - jlo) * BLOCK : (i - jlo + 1) * BLOCK],
                                start=(j == ilo),
                                stop=(j == ihi),
                            )
                        ob = sbo.tile([D + 1, BLOCK], f32, tag="ob", name=f"ob_{bh}_{i}")
                        nc.vector.tensor_copy(out=ob, in_=acc[:, :])
                        nc.sync.dma_start(out=out[bh, i], in_=ob)
    nc.compile()
    return nc


def _host_tensors(q, k, v):
    """Build the pre-transposed / augmented device input arrays.

    q,k,v: [BH, T, D] float32. Returns qt [BH,128,GUARD_NB*128] bf16 (qT
    duplicated on both partition halves, zero guard columns), kt [BH,64,T]
    bf16, va [BH,T,D+1] bf16 (ones column appended).
    """
    qtT = np.ascontiguousarray(q.transpose(0, 2, 1)).astype(_BF16)  # [BH, 64, T]
    ktT = np.ascontiguousarray(k.transpose(0, 2, 1)).astype(_BF16)
    qt = np.zeros((BH, 128, GUARD_NB * BLOCK), dtype=_BF16)
    qt[:, 0:64, BLOCK : BLOCK + T] = qtT
    qt[:, 64:128, BLOCK : BLOCK + T] = qtT
    va = np.empty((BH, T, D + 1), dtype=_BF16)
    va[:, :, :D] = v.astype(_BF16)
    va[:, :, D] = np.float32(1.0)
    return qt, ktT, va


def _epilogue(outT, q, k, v, mask):
    """outT: [BH, NB, D+1, BLOCK] device result. Returns [N,H,T,D] fp32."""
    # unnormalized local output [BH, T, D] and softmax sums [BH, T]
    o = outT[:, :, 0:D, :].transpose(0, 1, 3, 2).reshape(BH, T, D).copy()
    sums = outT[:, :, D, :].reshape(BH, T).copy()

    # BOS-token key slot for query blocks >= 2 (blocks 0/1 already have key 0
    # inside their local window, which equals the reference's global slot).
    k0 = k[:, 0, :]  # [BH, D]
    v0 = v[:, 0, :]
    qs = q[:, 2 * BLOCK :, :]  # queries 256..4095
    pk = np.exp(np.einsum("bqd,bd->bq", qs, k0) * SCALE + EXP_BIAS)
    o[:, 2 * BLOCK :, :] += pk[:, :, None] * v0[:, None, :]
    sums[:, 2 * BLOCK :] += pk

    o /= sums[:, :, None]

    # BOS query row: full attention of query 0 over all T keys.
    mrow = np.repeat(mask[:, 0, 0, :], H, axis=0)  # [BH, T]
    s0 = np.einsum("bd,btd->bt", q[:, 0, :], k) * SCALE + mrow
    s0 -= s0.max(axis=1, keepdims=True)
    p0 = np.exp(s0)
    p0 /= p0.sum(axis=1, keepdims=True)
    o[:, 0, :] = np.einsum("bt,btd->bd", p0, v)

    return o.reshape(N, H, T, D).astype(np.float32)


def kernel(query_layer, key_layer, value_layer, attention_mask):
    from concourse.bass_utils import run_bass_kernel_spmd

    q = np.asarray(query_layer, dtype=np.float32).reshape(BH, T, D)
    k = np.asarray(key_layer, dtype=np.float32).reshape(BH, T, D)
    v = np.asarray(value_layer, dtype=np.float32).reshape(BH, T, D)
    mask = np.asarray(attention_mask, dtype=np.float32)  # [N,1,1,T]

    qt, kt, va = _host_tensors(q, k, v)

    if "nc" not in _CACHE:
        _CACHE["nc"] = _build_bass()
    nc = _CACHE["nc"]

    in_maps = []
    for c in range(NCORES):
        s = slice(c * BH_PER_CORE, (c + 1) * BH_PER_CORE)
        in_maps.append({"qt": qt[s], "kt": kt[s], "va": va[s]})

    res = run_bass_kernel_spmd(nc, in_maps, core_ids=list(range(NCORES)))
    outT = np.concatenate(
        [r["out"].astype(np.float32) for r in res.results], axis=0
    )  # [BH, NB, D+1, BLOCK]
    return _epilogue(outT, q, k, v, mask)


# revision 19
# speedup vs baseline: 3.7554x; 3.7554x over previous
"""LSG (local-sparse-global) block-local self-attention for Trainium2.

Problem: n=2, h=16, t=4096, d=64, block=128. Each query block attends to a
3-block local key window (1-block halo each side) plus a global BOS token
slot; the BOS query (position 0) attends to everything.

Strategy (8 NeuronCores, batch*head = 32 sharded 4 per core):
  - Host pre-transposes Q/K to [d, t] bf16 layouts (K in a row-paired layout:
    even key blocks on partitions 0-63, odd on 64-127) and appends a
    ones-column to V so per-query softmax denominators ride along the PV
    matmul. One big DMA per tensor per batch*head.
  - Device computes, per key block j, S^T = kT_j.T @ qT_union in PSUM.
    Two key blocks run concurrently via PE row tiling (row groups 0-63 /
    64-127) against a shared 512-wide query union, qT duplicated on both
    partition halves.
  - Softmax uses no running max: p = exp(s/8). Scores/8 are ~N(0,1) so exp
    stays comfortably in fp32 range and any constant bias cancels after
    normalization. This removes the max-reduce and means exp output IS
    already P^T (keys on partitions), so the PV matmul needs no transpose.
  - out^T[d, q] (+ sums row 64) accumulates over the window key blocks.
  - Host divides by sums, adds the BOS-token key slot for query blocks >= 2
    (for blocks 0/1 key 0 is already inside the local window, which matches
    the reference's global-slot semantics exactly), and computes the single
    BOS query row. These host pieces are ~0.5% of total FLOPs.
"""

import sys

import numpy as np
import ml_dtypes

try:  # concourse (bass) ships in the trn_rl repo, not on the default path
    import concourse.bass  # noqa: F401
except ImportError:
    for _p in ("/opt/trn_rl_repo", "/root/.axon_site/_ro/trn_rl_repo"):
        if _p not in sys.path:
            sys.path.insert(0, _p)

N, H, T, D = 2, 16, 4096, 64
BLOCK = 128
NB = T // BLOCK            # 32 key/query blocks
NP = NB // 2               # 16 key-block pairs
BH = N * H                 # 32 batch*head pairs
NCORES = 8
BH_PER_CORE = BH // NCORES  # 4
GUARD_NB = NB + 3          # query column blocks incl. zero guards
EXP_BIAS = 0.0             # scores/8 ~ N(0,1): plain exp stays in fp32 range
SCALE = 1.0 / 8.0          # 1/sqrt(64)
OBATCH = 4                 # query blocks per output DMA

_BF16 = ml_dtypes.bfloat16

_CACHE = {}


def _build_bass():
    import concourse.bacc as bacc
    import concourse.mybir as mybir
    import concourse.tile as tile

    bf16 = mybir.dt.bfloat16
    f32 = mybir.dt.float32

    nc = bacc.Bacc(None, target_bir_lowering=False)
    qt = nc.declare_dram_parameter(
        "qt", [BH_PER_CORE, 128, GUARD_NB * BLOCK], bf16, isOutput=False
    )
    # kt: row-paired kT. [bh, 0:64, 128p:128(p+1)] = key block 2p (d-major),
    #     [bh, 64:128, ...] = key block 2p+1.
    kt = nc.declare_dram_parameter(
        "kt", [BH_PER_CORE, 128, NP * BLOCK], bf16, isOutput=False
    )
    # va: [bh, p, 65j:65j+65] = [v[128j + p, :], 1.0]
    va = nc.declare_dram_parameter(
        "va", [BH_PER_CORE, 128, NB * (D + 1)], bf16, isOutput=False
    )
    out = nc.declare_dram_parameter(
        "out", [BH_PER_CORE, NB // OBATCH, D + 1, OBATCH * BLOCK], f32, isOutput=True
    )

    with tile.TileContext(nc) as tc:
        with (
            tc.tile_pool(name="cst", bufs=1) as cst,
            tc.tile_pool(name="sbq", bufs=2) as sbq,
            tc.tile_pool(name="sbk", bufs=2) as sbk,
            tc.tile_pool(name="sbv", bufs=2) as sbv,
            tc.tile_pool(name="sbp", bufs=1) as sbp,
            tc.tile_pool(name="sbo", bufs=3) as sbo,
            tc.tile_pool(name="psS", bufs=2, space="PSUM") as psS,
            tc.tile_pool(name="psO", bufs=3, space="PSUM") as psO,
        ):
            bias_tile = cst.tile([128, 1], f32, tag="bias")
            nc.vector.memset(bias_tile, EXP_BIAS)
            # Touch the bias from ACT once so later Exp ops don't each carry
            # a cross-engine wait (the AC instruction has one wait slot).
            warm = cst.tile([128, 1], f32, tag="warm")
            nc.scalar.activation(
                out=warm,
                in_=bias_tile,
                func=mybir.ActivationFunctionType.Copy,
                bias=0.0,
                scale=1.0,
            )
            for bh in range(BH_PER_CORE):
                qta = sbq.tile([128, GUARD_NB * BLOCK], bf16, tag="qta")
                half = GUARD_NB * BLOCK // 2
                for c in range(2):
                    nc.sync.dma_start(
                        out=qta[:, c * half : (c + 1) * half],
                        in_=qt[bh, :, c * half : (c + 1) * half],
                    )
                kta = sbk.tile([128, NP * BLOCK], bf16, tag="kta")
                nc.sync.dma_start(out=kta, in_=kt[bh])
                vaa = sbv.tile([128, NB * (D + 1)], bf16, tag="vaa")
                nc.sync.dma_start(out=vaa, in_=va[bh])

                pts = {}
                obt = None
                for p in range(NP):
                    j0, j1 = 2 * p, 2 * p + 1
                    # scores^T for the pair. Each half gets its own 512-wide
                    # query union starting at its window's left edge (block
                    # j-1; qta includes the +1 guard-block shift), so the
                    # valid region is [0:384] in both psum halves.
                    u = j0 * BLOCK
                    sP = psS.tile([128, 1024], f32, tag="spair", name=f"sP_{bh}_{p}")
                    nc.tensor.matmul(
                        out=sP[:, 0:512],
                        lhsT=kta[0:64, p * BLOCK : (p + 1) * BLOCK],
                        rhs=qta[0:64, u : u + 512],
                        start=True,
                        stop=True,
                    )
                    nc.tensor.matmul(
                        out=sP[:, 512:1024],
                        lhsT=kta[64:128, p * BLOCK : (p + 1) * BLOCK],
                        rhs=qta[64:128, u + 128 : u + 640],
                        start=True,
                        stop=True,
                    )

                    # one exp over both halves' first 384 columns
                    # (guard-query columns exp to 1 and are never read)
                    ptp = sbp.tile(
                        [128, 2 * 3 * BLOCK],
                        bf16,
                        tag=f"ptp_{bh}_{p}",
                        name=f"pt_{bh}_{p}",
                    )
                    nc.scalar.activation(
                        out=ptp.rearrange("q (b w) -> q b w", b=2),
                        in_=sP.rearrange("q (b w) -> q b w", b=2)[:, :, 0:384],
                        func=mybir.ActivationFunctionType.Exp,
                        bias=bias_tile[:, :],
                        scale=SCALE,
                    )
                    pts[j0] = (ptp, j0 - 1, 0)
                    pts[j1] = (ptp, j0, 384)

                    # query blocks whose full window (keys i-1..i+1) is now
                    # available: run their accumulating matmuls back-to-back
                    # so each PSUM group opens and closes immediately.
                    if p == 0:
                        done = [0]
                    elif p == NP - 1:
                        done = [2 * p - 1, 2 * p, 2 * p + 1]
                    else:
                        done = [2 * p - 1, 2 * p]
                    for i in done:
                        ilo, ihi = max(0, i - 1), min(NB - 1, i + 1)
                        acc = psO.tile(
                            [D + 1, BLOCK], f32, tag="acc", name=f"acc_{bh}_{i}"
                        )
                        for j in range(ilo, ihi + 1):
                            ptj, jlo, off = pts[j]
                            base = off + (i - jlo) * BLOCK
                            nc.tensor.matmul(
                                out=acc[:, :],
                                lhsT=vaa[:, j * (D + 1) : (j + 1) * (D + 1)],
                                rhs=ptj[:, base : base + BLOCK],
                                start=(j == ilo),
                                stop=(j == ihi),
                            )
                        g, sl = i // OBATCH, i % OBATCH
                        if sl == 0:
                            obt = sbo.tile(
                                [D + 1, OBATCH * BLOCK],
                                f32,
                                tag="ob",
                                name=f"ob_{bh}_{g}",
                            )
                        nc.vector.tensor_copy(
                            out=obt[:, sl * BLOCK : (sl + 1) * BLOCK], in_=acc[:, :]
                        )
                        if sl == OBATCH - 1:
                            nc.sync.dma_start(out=out[bh, g], in_=obt)
    nc.compile()
    return nc


def _host_tensors(q, k, v):
    """Build the device input arrays from [BH, T, D] fp32 q/k/v.

    qt [BH,128,GUARD_NB*128]: qT duplicated on both partition halves with
        zero guard columns.
    kt [BH,128,NP*128]: kT row-paired (even key block on partitions 0-63,
        odd on 64-127).
    va [BH,128,NB*65]: per key block j, columns 65j..65j+64 hold
        [v[128j + p, :], 1.0] on partition p.
    """
    qtT = np.ascontiguousarray(q.transpose(0, 2, 1)).astype(_BF16)  # [BH, 64, T]
    ktT = np.ascontiguousarray(k.transpose(0, 2, 1)).astype(_BF16)
    qt = np.zeros((BH, 128, GUARD_NB * BLOCK), dtype=_BF16)
    qt[:, 0:64, BLOCK : BLOCK + T] = qtT
    qt[:, 64:128, BLOCK : BLOCK + T] = qtT

    ktb = ktT.reshape(BH, 64, NB, BLOCK)  # [BH, d, block j, col]
    kt = np.empty((BH, 128, NP * BLOCK), dtype=_BF16)
    kt[:, 0:64] = ktb[:, :, 0::2].reshape(BH, 64, NP * BLOCK)
    kt[:, 64:128] = ktb[:, :, 1::2].reshape(BH, 64, NP * BLOCK)

    va = np.empty((BH, 128, NB, D + 1), dtype=_BF16)
    va[:, :, :, :D] = v.reshape(BH, NB, BLOCK, D).transpose(0, 2, 1, 3)
    va[:, :, :, D] = np.float32(1.0)
    va = va.reshape(BH, 128, NB * (D + 1))
    return qt, kt, va


def _epilogue(outT, q, k, v, mask):
    """outT: [BH, NB//OBATCH, D+1, OBATCH*BLOCK] device result -> [N,H,T,D]."""
    outT = outT.reshape(BH, NB // OBATCH, D + 1, OBATCH, BLOCK)
    outT = outT.transpose(0, 1, 3, 2, 4).reshape(BH, NB, D + 1, BLOCK)
    # unnormalized local output [BH, T, D] and softmax sums [BH, T]
    o = outT[:, :, 0:D, :].transpose(0, 1, 3, 2).reshape(BH, T, D).copy()
    sums = outT[:, :, D, :].reshape(BH, T).copy()

    # BOS-token key slot for query blocks >= 2 (blocks 0/1 already have key 0
    # inside their local window, which equals the reference's global slot).
    k0 = k[:, 0, :]  # [BH, D]
    v0 = v[:, 0, :]
    qs = q[:, 2 * BLOCK :, :]  # queries 256..4095
    pk = np.exp(np.einsum("bqd,bd->bq", qs, k0) * SCALE + EXP_BIAS)
    o[:, 2 * BLOCK :, :] += pk[:, :, None] * v0[:, None, :]
    sums[:, 2 * BLOCK :] += pk

    o /= sums[:, :, None]

    # BOS query row: full attention of query 0 over all T keys.
    mrow = np.repeat(mask[:, 0, 0, :], H, axis=0)  # [BH, T]
    s0 = np.einsum("bd,btd->bt", q[:, 0, :], k) * SCALE + mrow
    s0 -= s0.max(axis=1, keepdims=True)
    p0 = np.exp(s0)
    p0 /= p0.sum(axis=1, keepdims=True)
    o[:, 0, :] = np.einsum("bt,btd->bd", p0, v)

    return o.reshape(N, H, T, D).astype(np.float32)


def kernel(query_layer, key_layer, value_layer, attention_mask):
    from concourse.bass_utils import run_bass_kernel_spmd

    q = np.asarray(query_layer, dtype=np.float32).reshape(BH, T, D)
    k = np.asarray(key_layer, dtype=np.float32).reshape(BH, T, D)
    v = np.asarray(value_layer, dtype=np.float32).reshape(BH, T, D)
    mask = np.asarray(attention_mask, dtype=np.float32)  # [N,1,1,T]

    qt, kt, va = _host_tensors(q, k, v)

    if "nc" not in _CACHE:
        _CACHE["nc"] = _build_bass()
    nc = _CACHE["nc"]

    in_maps = []
    for c in range(NCORES):
        s = slice(c * BH_PER_CORE, (c + 1) * BH_PER_CORE)
        in_maps.append({"qt": qt[s], "kt": kt[s], "va": va[s]})

    res = run_bass_kernel_spmd(nc, in_maps, core_ids=list(range(NCORES)))
    outT = np.concatenate(
        [r["out"].astype(np.float32) for r in res.results], axis=0
    )
    return _epilogue(outT, q, k, v, mask)
